# revision 24
# baseline (speedup 1.0000x reference)
"""Dropless grouped GEMM (MoE dispatch -> swiglu MLP -> combine) on 8 TRN2 cores.

Strategy (expert-parallel): host computes the FCFS slot assignment and the
token dispatch (the "all-to-all"), core e runs the full swiglu MLP for expert
e on its padded activation block, host combines the outputs back to token
order.

Because slot assignment is first-come-first-served, each expert's valid
tokens occupy slots [0, count_e) contiguously -- so the device only has to
compute the first ceil(max_count/128)*128 slots (c_eff), not the full
capacity C. The kernel is compiled per c_eff value (cached).

Device layout: everything is kept transposed so both GEMMs contract over the
partition dim with weights in their natural DRAM layout:
  GEMM1: out1T[f, t] = w1[h, f].T @ xT[h, t]      (lhsT = w1 slice)
  swiglu: hT[f, t] = sigmoid(gT)*gT*uT            (ACT + DVE, PSUM -> SBUF)
  GEMM2: outT[ho, t] = w2[f, ho].T @ hT[f, t]     (lhsT = w2 slice)
Matmuls run in bf16 with fp32 PSUM accumulation (full PE rate, FWL weight
loads); swiglu runs in fp32 out of PSUM with a single bf16 rounding into hT.
"""

import math
import ml_dtypes
import numpy as np
from contextlib import ExitStack

import concourse.bacc as bacc
import concourse.mybir as mybir
import concourse.tile as tile
from concourse.bass_utils import run_bass_kernel_spmd

E, C, H, F = 8, 768, 1024, 4096
F2 = 2 * F
P = 128
KH = H // P        # 8 contraction chunks for GEMM1
KF = F // P        # 32 contraction chunks for GEMM2
NG = F // P        # 32 gate/up f-tile pairs
NO = H // P        # 8 output H tiles
F32 = mybir.dt.float32
F32R = mybir.dt.float32r
BF16 = mybir.dt.bfloat16
ALU_MULT = mybir.AluOpType.mult
SIGMOID = mybir.ActivationFunctionType.Sigmoid

_NC_CACHE = {}


def _token_chunks(c_eff):
    """Split the token dim into PSUM-bank-sized (<=512) chunks."""
    if c_eff <= 512:
        return [(0, c_eff)]
    h = (c_eff // 2 + 15) // 16 * 16
    return [(0, h), (h, c_eff - h)]


def _expert_mlp_body(ctx, tc, xT, w1, w2, outT, c_eff):
    nc = tc.nc
    chunks = _token_chunks(c_eff)
    psum_bufs = 2 if len(chunks) == 2 else 4

    xpool = ctx.enter_context(tc.tile_pool(name="x", bufs=1))
    w1pool = ctx.enter_context(tc.tile_pool(name="w1", bufs=4))
    hpool = ctx.enter_context(tc.tile_pool(name="h", bufs=1))
    w2pool = ctx.enter_context(tc.tile_pool(name="w2", bufs=4))
    spool = ctx.enter_context(tc.tile_pool(name="s", bufs=4))
    opool = ctx.enter_context(tc.tile_pool(name="o", bufs=3))
    psum = ctx.enter_context(tc.tile_pool(name="ps", bufs=psum_bufs, space="PSUM"))

    # PE warm-up: ~40 dummy matmuls on a zeroed tile run while the first
    # DMAs are still in flight, so the HAM clock gate is already at 8/8
    # (2.4 GHz) when the real matmul stream starts. The scratch PSUM chain
    # uses a pg0 slot and is released before the second real chain needs it.
    warm = xpool.tile([P, 512], BF16, tag="warm")
    nc.vector.memset(warm[:], 0.0)
    pwarm = psum.tile([P, 512], F32, tag="pg0", name="pwarm", space="PSUM")
    NW = 40
    for w in range(NW):
        nc.tensor.matmul(
            pwarm[:], lhsT=warm[:, :P], rhs=warm[:], start=(w == 0), stop=(w == NW - 1)
        )

    # Startup-latency-critical DMA order: interleave the xT k-chunks with
    # per-k slices of the first gate/up weight group so the first
    # accumulation chain can begin as soon as its k=0 operands land.
    # W1 is streamed in groups of GW f-tiles so each DMA reads GW*512B
    # contiguous runs instead of 512B.
    GW = 4
    xt = xpool.tile([P, KH, c_eff], BF16, tag="xT")
    wg0 = w1pool.tile([P, KH, GW * P], BF16, tag="wg")
    wu0 = w1pool.tile([P, KH, GW * P], BF16, tag="wu")
    c00, c0n = _token_chunks(c_eff)[0]
    nc.sync.dma_start(out=xt[:, 0, :c0n], in_=xT[0:P, :c0n])
    if c0n < c_eff:
        nc.sync.dma_start(out=xt[:, 0, c0n:], in_=xT[0:P, c0n:c_eff])
    for k in range(KH):
        nc.sync.dma_start(out=wg0[:, k, :], in_=w1[k * P : (k + 1) * P, 0 : GW * P])
        nc.sync.dma_start(
            out=wu0[:, k, :], in_=w1[k * P : (k + 1) * P, F : F + GW * P]
        )
        if k + 1 < KH:
            nc.sync.dma_start(
                out=xt[:, k + 1, :], in_=xT[(k + 1) * P : (k + 2) * P, :c_eff]
            )

    # ---- GEMM1 + swiglu: hT[f, t], kept resident in SBUF ----
    h_tiles = []
    for i in range(NG):
        gi, sub = divmod(i, GW)
        if sub == 0:
            if i == 0:
                wgg, wug = wg0, wu0
            else:
                wgg = w1pool.tile([P, KH, GW * P], BF16, tag="wg", name=f"wg{gi}")
                wug = w1pool.tile([P, KH, GW * P], BF16, tag="wu", name=f"wu{gi}")
                f0 = gi * GW * P
                nc.sync.dma_start(
                    out=wgg[:],
                    in_=w1[:, f0 : f0 + GW * P].rearrange("(k p) f -> p k f", p=P),
                )
                nc.sync.dma_start(
                    out=wug[:],
                    in_=w1[:, F + f0 : F + f0 + GW * P].rearrange(
                        "(k p) f -> p k f", p=P
                    ),
                )
        wg = wgg[:, :, sub * P : (sub + 1) * P]
        wu = wug[:, :, sub * P : (sub + 1) * P]
        ht = hpool.tile([P, c_eff], BF16, tag=f"h{i}")
        # both token chunks run back-to-back under the same lhsT so the PE
        # can pipeline the (f32r-internal) weight load across them
        ps_g = [psum.tile([P, cn], F32, tag=f"pg{ci}", name=f"ps_g{i}_{ci}", space="PSUM")
                for ci, (c0, cn) in enumerate(chunks)]
        ps_u = [psum.tile([P, cn], F32, tag=f"pu{ci}", name=f"ps_u{i}_{ci}", space="PSUM")
                for ci, (c0, cn) in enumerate(chunks)]
        for k in range(KH):
            for ci, (c0, cn) in enumerate(chunks):
                nc.tensor.matmul(
                    ps_g[ci][:],
                    lhsT=wg[:, k, :],
                    rhs=xt[:, k, c0 : c0 + cn],
                    start=(k == 0),
                    stop=(k == KH - 1),
                )
            for ci, (c0, cn) in enumerate(chunks):
                nc.tensor.matmul(
                    ps_u[ci][:],
                    lhsT=wu[:, k, :],
                    rhs=xt[:, k, c0 : c0 + cn],
                    start=(k == 0),
                    stop=(k == KH - 1),
                )
        for ci, (c0, cn) in enumerate(chunks):
            tsl = slice(c0, c0 + cn)
            sg = spool.tile([P, cn], F32, tag=f"silu{ci}")
            nc.scalar.activation(sg[:], ps_g[ci][:], SIGMOID)
            nc.vector.tensor_tensor(out=sg[:], in0=sg[:], in1=ps_g[ci][:], op=ALU_MULT)
            nc.vector.tensor_tensor(
                out=ht[:, tsl], in0=sg[:], in1=ps_u[ci][:], op=ALU_MULT
            )
        h_tiles.append(ht)

    # ---- GEMM2: outT[ho, t] = sum_k w2[k-chunk, ho-tile].T @ hT[k-chunk, t] ----
    for j in range(NO):
        w2t = w2pool.tile([P, KF, P], BF16, tag="w2", name=f"w2_{j}")
        nc.sync.dma_start(
            out=w2t[:],
            in_=w2[:, j * P : (j + 1) * P].rearrange("(k p) f -> p k f", p=P),
        )
        ot = opool.tile([P, c_eff], F32, tag="o", name=f"ot{j}")
        po = [psum.tile([P, cn], F32, tag=f"pg{ci}", name=f"po{j}_{ci}", space="PSUM")
              for ci, (c0, cn) in enumerate(chunks)]
        for k in range(KF):
            for ci, (c0, cn) in enumerate(chunks):
                nc.tensor.matmul(
                    po[ci][:],
                    lhsT=w2t[:, k, :],
                    rhs=h_tiles[k][:, c0 : c0 + cn],
                    start=(k == 0),
                    stop=(k == KF - 1),
                )
        for ci, (c0, cn) in enumerate(chunks):
            nc.vector.tensor_copy(out=ot[:, c0 : c0 + cn], in_=po[ci][:])
        nc.sync.dma_start(out=outT[j * P : (j + 1) * P, :c_eff], in_=ot[:])


def _build_nc(c_eff):
    nc = bacc.Bacc(trn_type="TRN2")
    xT = nc.dram_tensor("xT", [H, C], BF16, kind="ExternalInput")
    w1 = nc.dram_tensor("w1", [H, F2], BF16, kind="ExternalInput")
    w2 = nc.dram_tensor("w2", [F, H], BF16, kind="ExternalInput")
    outT = nc.dram_tensor("outT", [H, C], F32, kind="ExternalOutput")
    with tile.TileContext(nc) as tc:
        with ExitStack() as ctx:
            _expert_mlp_body(ctx, tc, xT.ap(), w1.ap(), w2.ap(), outT.ap(), c_eff)
    nc.finalize()
    return nc


def _get_nc(c_eff):
    if c_eff not in _NC_CACHE:
        _NC_CACHE[c_eff] = _build_nc(c_eff)
    return _NC_CACHE[c_eff]


def _assign(expert_ids):
    """FCFS slot assignment: offset[t] = # earlier tokens with same expert."""
    T = expert_ids.shape[0]
    eid = np.clip(expert_ids, 0, E - 1).astype(np.int64)
    counts = np.bincount(eid, minlength=E)
    starts = np.concatenate([[0], np.cumsum(counts)[:-1]])
    order = np.argsort(eid, kind="stable")
    offset = np.empty(T, dtype=np.int64)
    offset[order] = np.arange(T, dtype=np.int64) - starts[eid[order]]
    return eid, offset, counts


def _prepare(tokens, expert_ids):
    """Host-side routing + dispatch. Returns (xT [E,H,C], slot, valid, c_eff)."""
    eid, offset, counts = _assign(expert_ids)
    valid = offset < C
    slot = np.where(valid, eid * C + offset, E * C)
    # tokens only ever appear as the matmul free dim, so c_eff needs no
    # 128-alignment; pad to 16 for DMA-friendly strides
    c_eff = int(min(C, math.ceil(max(int(counts.max()), 1) / 16) * 16))

    padded = np.zeros((E * C + 1, H), np.float32)
    padded[slot] = tokens
    xT = np.ascontiguousarray(
        padded[: E * C].reshape(E, C, H).transpose(0, 2, 1).astype(ml_dtypes.bfloat16)
    )
    return xT, slot, valid, c_eff


def kernel(tokens, expert_ids, w_gate_up, w_down, **_unused):
    tokens = np.ascontiguousarray(np.asarray(tokens), dtype=np.float32)
    expert_ids = np.asarray(expert_ids)
    w_gate_up = np.ascontiguousarray(
        np.asarray(w_gate_up, dtype=np.float32).astype(ml_dtypes.bfloat16)
    )
    w_down = np.ascontiguousarray(
        np.asarray(w_down, dtype=np.float32).astype(ml_dtypes.bfloat16)
    )

    xT, slot, valid, c_eff = _prepare(tokens, expert_ids)

    nc = _get_nc(c_eff)
    in_maps = [
        {"xT": xT[e], "w1": w_gate_up[e], "w2": w_down[e]} for e in range(E)
    ]
    res = run_bass_kernel_spmd(nc, in_maps, core_ids=list(range(E)))

    outT = np.stack([res.results[e]["outT"] for e in range(E)])  # [E, H, C]
    out_ec = outT.transpose(0, 2, 1).reshape(E * C, H)
    gathered = out_ec[np.clip(slot, 0, E * C - 1)]
    output = np.where(valid[:, None], gathered, np.float32(0.0)).astype(np.float32)
    return output, valid


# revision 26
# speedup vs baseline: 1.0357x; 1.0357x over previous
"""Dropless grouped GEMM (MoE dispatch -> swiglu MLP -> combine) on 8 TRN2 cores.

Strategy (expert-parallel): host computes the FCFS slot assignment and the
token dispatch (the "all-to-all"), core e runs the full swiglu MLP for expert
e on its padded activation block, host combines the outputs back to token
order.

Because slot assignment is first-come-first-served, each expert's valid
tokens occupy slots [0, count_e) contiguously -- so the device only has to
compute the first c_eff = ceil(max_count/16)*16 slots, not the full
capacity C (tokens only appear as the matmul free dim, so no 128-alignment
is needed). The kernel is compiled per c_eff value (cached).

Device layout: everything is kept transposed so both GEMMs contract over the
partition dim with weights in their natural DRAM layout:
  GEMM1: out1T[f, t] = w1[h, f].T @ xT[h, t]      (lhsT = w1 slice)
  swiglu: hT[f, t] = sigmoid(gT)*gT*uT            (ACT + DVE, PSUM -> SBUF)
  GEMM2: outT[ho, t] = w2[f, ho].T @ hT[f, t]     (lhsT = w2 slice)
Matmuls run in bf16 with fp32 PSUM accumulation (full PE rate, FWL weight
loads); swiglu runs in fp32 out of PSUM with a single bf16 rounding into hT.
"""

import math
import ml_dtypes
import numpy as np
from contextlib import ExitStack

import concourse.bacc as bacc
import concourse.mybir as mybir
import concourse.tile as tile
from concourse.bass_utils import run_bass_kernel_spmd

E, C, H, F = 8, 768, 1024, 4096
F2 = 2 * F
P = 128
KH = H // P        # 8 contraction chunks for GEMM1
KF = F // P        # 32 contraction chunks for GEMM2
NG = F // P        # 32 gate/up f-tile pairs
NO = H // P        # 8 output H tiles
F32 = mybir.dt.float32
F32R = mybir.dt.float32r
BF16 = mybir.dt.bfloat16
ALU_MULT = mybir.AluOpType.mult
SIGMOID = mybir.ActivationFunctionType.Sigmoid

_NC_CACHE = {}


def _token_chunks(c_eff):
    """Split the token dim into PSUM-bank-sized (<=512) chunks."""
    if c_eff <= 512:
        return [(0, c_eff)]
    h = (c_eff // 2 + 15) // 16 * 16
    return [(0, h), (h, c_eff - h)]


def _expert_mlp_body(ctx, tc, xT, w1, w2, outT, c_eff):
    nc = tc.nc
    chunks = _token_chunks(c_eff)
    psum_bufs = 2 if len(chunks) == 2 else 4

    xpool = ctx.enter_context(tc.tile_pool(name="x", bufs=1))
    w1pool = ctx.enter_context(tc.tile_pool(name="w1", bufs=4))
    hpool = ctx.enter_context(tc.tile_pool(name="h", bufs=1))
    w2pool = ctx.enter_context(tc.tile_pool(name="w2", bufs=4))
    spool = ctx.enter_context(tc.tile_pool(name="s", bufs=4))
    opool = ctx.enter_context(tc.tile_pool(name="o", bufs=3))
    psum = ctx.enter_context(tc.tile_pool(name="ps", bufs=psum_bufs, space="PSUM"))

    # Startup-latency-critical DMA order: interleave the xT k-chunks with
    # per-k slices of the first gate/up weight group so the first
    # accumulation chain can begin as soon as its k=0 operands land.
    # W1 is streamed in groups of GW f-tiles so each DMA reads GW*512B
    # contiguous runs instead of 512B.
    GW = 4
    xt = xpool.tile([P, KH, c_eff], BF16, tag="xT")
    wg0 = w1pool.tile([P, KH, GW * P], BF16, tag="wg")
    wu0 = w1pool.tile([P, KH, GW * P], BF16, tag="wu")
    c00, c0n = _token_chunks(c_eff)[0]
    nc.sync.dma_start(out=xt[:, 0, :c0n], in_=xT[0:P, :c0n])
    if c0n < c_eff:
        nc.sync.dma_start(out=xt[:, 0, c0n:], in_=xT[0:P, c0n:c_eff])
    for k in range(KH):
        nc.sync.dma_start(out=wg0[:, k, :], in_=w1[k * P : (k + 1) * P, 0 : GW * P])
        nc.sync.dma_start(
            out=wu0[:, k, :], in_=w1[k * P : (k + 1) * P, F : F + GW * P]
        )
        if k + 1 < KH:
            nc.sync.dma_start(
                out=xt[:, k + 1, :], in_=xT[(k + 1) * P : (k + 2) * P, :c_eff]
            )

    # ---- GEMM1 + swiglu: hT[f, t], kept resident in SBUF ----
    h_tiles = []
    for i in range(NG):
        gi, sub = divmod(i, GW)
        if sub == 0:
            if i == 0:
                wgg, wug = wg0, wu0
            else:
                wgg = w1pool.tile([P, KH, GW * P], BF16, tag="wg", name=f"wg{gi}")
                wug = w1pool.tile([P, KH, GW * P], BF16, tag="wu", name=f"wu{gi}")
                f0 = gi * GW * P
                nc.sync.dma_start(
                    out=wgg[:],
                    in_=w1[:, f0 : f0 + GW * P].rearrange("(k p) f -> p k f", p=P),
                )
                nc.sync.dma_start(
                    out=wug[:],
                    in_=w1[:, F + f0 : F + f0 + GW * P].rearrange(
                        "(k p) f -> p k f", p=P
                    ),
                )
        wg = wgg[:, :, sub * P : (sub + 1) * P]
        wu = wug[:, :, sub * P : (sub + 1) * P]
        ht = hpool.tile([P, c_eff], BF16, tag=f"h{i}")
        # both token chunks run back-to-back under the same lhsT so the PE
        # can pipeline the (f32r-internal) weight load across them
        ps_g = [psum.tile([P, cn], F32, tag=f"pg{ci}", name=f"ps_g{i}_{ci}", space="PSUM")
                for ci, (c0, cn) in enumerate(chunks)]
        ps_u = [psum.tile([P, cn], F32, tag=f"pu{ci}", name=f"ps_u{i}_{ci}", space="PSUM")
                for ci, (c0, cn) in enumerate(chunks)]
        for k in range(KH):
            for ci, (c0, cn) in enumerate(chunks):
                nc.tensor.matmul(
                    ps_g[ci][:],
                    lhsT=wg[:, k, :],
                    rhs=xt[:, k, c0 : c0 + cn],
                    start=(k == 0),
                    stop=(k == KH - 1),
                )
            for ci, (c0, cn) in enumerate(chunks):
                nc.tensor.matmul(
                    ps_u[ci][:],
                    lhsT=wu[:, k, :],
                    rhs=xt[:, k, c0 : c0 + cn],
                    start=(k == 0),
                    stop=(k == KH - 1),
                )
        for ci, (c0, cn) in enumerate(chunks):
            tsl = slice(c0, c0 + cn)
            sg = spool.tile([P, cn], F32, tag=f"silu{ci}")
            nc.scalar.activation(sg[:], ps_g[ci][:], SIGMOID)
            nc.vector.tensor_tensor(out=sg[:], in0=sg[:], in1=ps_g[ci][:], op=ALU_MULT)
            nc.vector.tensor_tensor(
                out=ht[:, tsl], in0=sg[:], in1=ps_u[ci][:], op=ALU_MULT
            )
        h_tiles.append(ht)

    # ---- GEMM2: outT[ho, t] = sum_k w2[k-chunk, ho-tile].T @ hT[k-chunk, t] ----
    for j in range(NO):
        w2t = w2pool.tile([P, KF, P], BF16, tag="w2", name=f"w2_{j}")
        nc.sync.dma_start(
            out=w2t[:],
            in_=w2[:, j * P : (j + 1) * P].rearrange("(k p) f -> p k f", p=P),
        )
        ot = opool.tile([P, c_eff], F32, tag="o", name=f"ot{j}")
        po = [psum.tile([P, cn], F32, tag=f"pg{ci}", name=f"po{j}_{ci}", space="PSUM")
              for ci, (c0, cn) in enumerate(chunks)]
        for k in range(KF):
            for ci, (c0, cn) in enumerate(chunks):
                nc.tensor.matmul(
                    po[ci][:],
                    lhsT=w2t[:, k, :],
                    rhs=h_tiles[k][:, c0 : c0 + cn],
                    start=(k == 0),
                    stop=(k == KF - 1),
                )
        for ci, (c0, cn) in enumerate(chunks):
            nc.vector.tensor_copy(out=ot[:, c0 : c0 + cn], in_=po[ci][:])
        nc.sync.dma_start(out=outT[j * P : (j + 1) * P, :c_eff], in_=ot[:])


def _build_nc(c_eff):
    nc = bacc.Bacc(trn_type="TRN2")
    xT = nc.dram_tensor("xT", [H, C], BF16, kind="ExternalInput")
    w1 = nc.dram_tensor("w1", [H, F2], BF16, kind="ExternalInput")
    w2 = nc.dram_tensor("w2", [F, H], BF16, kind="ExternalInput")
    outT = nc.dram_tensor("outT", [H, C], F32, kind="ExternalOutput")
    with tile.TileContext(nc) as tc:
        with ExitStack() as ctx:
            _expert_mlp_body(ctx, tc, xT.ap(), w1.ap(), w2.ap(), outT.ap(), c_eff)
    nc.finalize()
    return nc


def _get_nc(c_eff):
    if c_eff not in _NC_CACHE:
        _NC_CACHE[c_eff] = _build_nc(c_eff)
    return _NC_CACHE[c_eff]


def _assign(expert_ids):
    """FCFS slot assignment: offset[t] = # earlier tokens with same expert."""
    T = expert_ids.shape[0]
    eid = np.clip(expert_ids, 0, E - 1).astype(np.int64)
    counts = np.bincount(eid, minlength=E)
    starts = np.concatenate([[0], np.cumsum(counts)[:-1]])
    order = np.argsort(eid, kind="stable")
    offset = np.empty(T, dtype=np.int64)
    offset[order] = np.arange(T, dtype=np.int64) - starts[eid[order]]
    return eid, offset, counts


def _prepare(tokens, expert_ids):
    """Host-side routing + dispatch. Returns (xT [E,H,C], slot, valid, c_eff)."""
    eid, offset, counts = _assign(expert_ids)
    valid = offset < C
    slot = np.where(valid, eid * C + offset, E * C)
    # tokens only ever appear as the matmul free dim, so c_eff needs no
    # 128-alignment; pad to 16 for DMA-friendly strides
    c_eff = int(min(C, math.ceil(max(int(counts.max()), 1) / 16) * 16))

    padded = np.zeros((E * C + 1, H), np.float32)
    padded[slot] = tokens
    xT = np.ascontiguousarray(
        padded[: E * C].reshape(E, C, H).transpose(0, 2, 1).astype(ml_dtypes.bfloat16)
    )
    return xT, slot, valid, c_eff


def kernel(tokens, expert_ids, w_gate_up, w_down, **_unused):
    tokens = np.ascontiguousarray(np.asarray(tokens), dtype=np.float32)
    expert_ids = np.asarray(expert_ids)
    w_gate_up = np.ascontiguousarray(
        np.asarray(w_gate_up, dtype=np.float32).astype(ml_dtypes.bfloat16)
    )
    w_down = np.ascontiguousarray(
        np.asarray(w_down, dtype=np.float32).astype(ml_dtypes.bfloat16)
    )

    xT, slot, valid, c_eff = _prepare(tokens, expert_ids)

    nc = _get_nc(c_eff)
    in_maps = [
        {"xT": xT[e], "w1": w_gate_up[e], "w2": w_down[e]} for e in range(E)
    ]
    res = run_bass_kernel_spmd(nc, in_maps, core_ids=list(range(E)))

    outT = np.stack([res.results[e]["outT"] for e in range(E)])  # [E, H, C]
    out_ec = outT.transpose(0, 2, 1).reshape(E * C, H)
    gathered = out_ec[np.clip(slot, 0, E * C - 1)]
    output = np.where(valid[:, None], gathered, np.float32(0.0)).astype(np.float32)
    return output, valid
